# revision 12
# baseline (speedup 1.0000x reference)
"""Multi-head attention (B=8, S=2048, D=512, H=8, DH=64) on 8 TRN2 NeuronCores.

Strategy: data-parallel over the batch dim — core b computes batch element b
end-to-end (no collectives). Per core, everything is kept transposed
("feature on partitions") so that softmax reductions land on the TensorE
contraction axis. The kernel is ACT-bound (exp over S^2 per head), so the
schedule keeps the Activation engine saturated from ~25us onward:

  1. Lead-in: load x/wq/wk, junk-warm the PE, project Q/K for head pair 0
     and V chunks 0..7, then enter the head loop immediately.
  2. Remaining QKV projection work (V chunks 8..15, Q/K for head pairs
     1..3) is injected into the attention loop as small 4-matmul "filler
     pieces" on alternating j-chunks, sized to fit the PE slack under the
     exp stream (ACT: ~2390ns/jc, PE: scores pair 518 + PV 1036).
  3. Scores are computed transposed via row-packed K=64 matmul pairs into
     disjoint PE row groups; exp(scale*S^T) runs on ScalarE out of PSUM.
  4. O^T = Vaug E^T accumulates over j with a ones-row denominator.
  5. Heads 0-6 normalize via reciprocal + DRAM partition-broadcast DMA
     (fully hidden under the next head's exp stream). Head 7 (critical
     tail) normalizes on-chip: DVE reciprocal of the PSUM denominator row,
     GpSimd partition_broadcast, DVE multiply straight out of PSUM; junk
     matmuls keep the PE hot so the output projection runs at full clock.
"""

import numpy as np
import ml_dtypes

B, S, D = 8, 2048, 512
H, DH = 8, 64
INNER = H * DH
SCALE = DH ** -0.5

N_CORES = 8
NDT = D // 128   # 4 contraction tiles
NSC = S // 128   # 16 s-chunks (j-chunks)
NST = S // 512   # 4 s-tiles


def _build_kernel():
    import concourse.bass as bass
    import concourse.mybir as mybir
    import concourse.tile as tile
    from concourse import bacc

    bf16 = mybir.dt.bfloat16
    f32 = mybir.dt.float32
    Exp = mybir.ActivationFunctionType.Exp

    nc = bacc.Bacc()

    xT = nc.declare_dram_parameter("xT", [D, S], bf16, isOutput=False)
    wq = nc.declare_dram_parameter("wq", [D, INNER], bf16, isOutput=False)
    wk = nc.declare_dram_parameter("wk", [D, INNER], bf16, isOutput=False)
    wv = nc.declare_dram_parameter("wv", [D, INNER], bf16, isOutput=False)
    wo = nc.declare_dram_parameter("wo", [INNER, D], bf16, isOutput=False)
    bo = nc.declare_dram_parameter("bo", [NDT, 128, 1], f32, isOutput=False)
    out = nc.declare_dram_parameter("out", [D, S], f32, isOutput=True)
    den_dram = nc.dram_tensor("den_scratch", [H, S], f32)

    with tile.TileContext(nc) as tc:
        with (
            tc.tile_pool(name="weights", bufs=1) as wpool,
            tc.tile_pool(name="acts", bufs=1) as apool,
            tc.tile_pool(name="et", bufs=4) as epool,
            tc.tile_pool(name="small", bufs=2) as spool,
            tc.tile_pool(name="ostage", bufs=2) as opool,
            tc.tile_pool(name="psA", bufs=2, space="PSUM") as psA,
            tc.tile_pool(name="psV", bufs=1, space="PSUM") as psV,
        ):
            # ---- load inputs (x and q/k weights first: they gate head 0) ----
            xT_s = [[wpool.tile([128, S // 2], bf16, name=f"xT{d}_{hf}",
                              tag=f"xT{d}_{hf}") for hf in range(2)]
                    for d in range(NDT)]
            wq_s = [wpool.tile([128, INNER], bf16, name=f"wq{d}", tag=f"wq{d}")
                    for d in range(NDT)]
            wk_s = [wpool.tile([128, INNER], bf16, name=f"wk{d}", tag=f"wk{d}")
                    for d in range(NDT)]
            wv_s = [wpool.tile([128, INNER], bf16, name=f"wv{d}", tag=f"wv{d}")
                    for d in range(NDT)]
            wo_s = [wpool.tile([128, D], bf16, name=f"wo{d}", tag=f"wo{d}")
                    for d in range(NDT)]
            bo_s = [wpool.tile([128, 1], f32, name=f"bo{d}", tag=f"bo{d}")
                    for d in range(NDT)]
            for d in range(NDT):
                sl = slice(d * 128, (d + 1) * 128)
                nc.sync.dma_start(out=xT_s[d][0][:], in_=xT[sl, 0:S // 2])
                nc.sync.dma_start(out=wq_s[d][:], in_=wq[sl, :])
                nc.scalar.dma_start(out=xT_s[d][1][:], in_=xT[sl, S // 2:])
                nc.scalar.dma_start(out=wk_s[d][:], in_=wk[sl, :])
                nc.gpsimd.dma_start(out=wv_s[d][:], in_=wv[sl, :])
            for d in range(NDT):
                sl = slice(d * 128, (d + 1) * 128)
                nc.gpsimd.dma_start(out=wo_s[d][:], in_=wo[sl, :])
                nc.gpsimd.dma_start(out=bo_s[d][:], in_=bo[d, :, :])

            # ---- QKV target tiles ----
            qt_lo = [apool.tile([128, S], bf16, name=f"qlo{t}", tag=f"qlo{t}")
                     for t in range(NDT)]
            kt_lo = [apool.tile([128, S], bf16, name=f"klo{t}", tag=f"klo{t}")
                     for t in range(NDT)]
            qt_hi = [apool.tile([128, S], bf16, name=f"qhi{t}", tag=f"qhi{t}")
                     for t in range(NDT)]
            kt_hi = [apool.tile([128, S], bf16, name=f"khi{t}", tag=f"khi{t}")
                     for t in range(NDT)]
            v_aug = [apool.tile([128, H * (DH + 1)], bf16, name=f"va{m}",
                                tag=f"va{m}") for m in range(NSC)]

            # PE warm-up: junk matmuls during the input-DMA window keep the
            # HAM activity monitor busy so real matmuls start fast.
            junk_sb = wpool.tile([128, 512], bf16, name="junk", tag="junk")
            nc.vector.memset(junk_sb[:, :], 0.0)
            # Trigger the GpSimd ucode library load now (it is lazy); the
            # head-7 partition_broadcast would otherwise eat a ~7us
            # LOAD_LIB on the critical tail.
            pre_bc = wpool.tile([2, 16], f32, name="prebc", tag="prebc")
            nc.vector.memset(pre_bc[:, :], 1.0)
            nc.gpsimd.partition_broadcast(pre_bc[:, :], pre_bc[0:1, :],
                                          channels=2)
            junk_ps = psV.tile([128, 4 * 512], f32, name="junkps", tag="pv")
            for k in range(6):
                nc.tensor.matmul(
                    junk_ps[:, (k % 4) * 512:(k % 4 + 1) * 512],
                    lhsT=junk_sb[:, 0:128],
                    rhs=junk_sb[:, :],
                )

            def qk_chunk(w_s, dst, ch, eng):
                """Full Q or K projection for head pair ch (lead-in only)."""
                for half in range(2):
                    pa = psA.tile([128, 1024], f32, name="pa", tag="pa")
                    for d in range(NDT):
                        for nn in range(2):
                            s0 = nn * 512
                            nc.tensor.matmul(
                                pa[:, nn * 512:(nn + 1) * 512],
                                lhsT=w_s[d][:, ch * 128:(ch + 1) * 128],
                                rhs=xT_s[d][half][:, s0:s0 + 512],
                                start=(d == 0),
                                stop=(d == NDT - 1),
                            )
                    eng.tensor_copy(
                        dst[ch][:, half * 1024:(half + 1) * 1024], pa[:, :])

            def qk_piece(w_s, dst, ch, half, nn):
                """4-matmul filler piece: one 512-col block of a projection."""
                pa = psA.tile([128, 1024], f32, name="pa", tag="pa")
                s0 = nn * 512
                for d in range(NDT):
                    nc.tensor.matmul(
                        pa[:, 0:512],
                        lhsT=w_s[d][:, ch * 128:(ch + 1) * 128],
                        rhs=xT_s[d][half][:, s0:s0 + 512],
                        start=(d == 0),
                        stop=(d == NDT - 1),
                    )
                off = half * 1024 + nn * 512
                nc.vector.tensor_copy(dst[ch][:, off:off + 512], pa[:, 0:512])

            def swap_half(src_lo, src_hi, t, sh):
                """DMA-swap one 1024-wide s-half of the lo replica into hi."""
                s0, s1 = sh * 1024, (sh + 1) * 1024
                nc.sync.dma_start(out=src_hi[t][64:128, s0:s1],
                                  in_=src_lo[t][0:64, s0:s1])
                nc.sync.dma_start(out=src_hi[t][0:64, s0:s1],
                                  in_=src_lo[t][64:128, s0:s1])

            def v_round(r):
                """V projection for j-chunks 4r..4r+3 (lead-in only)."""
                pvt = psV.tile([128, 4 * 512], f32, name="pvt", tag="pv")
                for k in range(4):
                    m = 4 * r + k
                    for d in range(NDT):
                        mh, mo = divmod(m, 8)
                        nc.tensor.matmul(
                            pvt[:, k * 512:(k + 1) * 512],
                            lhsT=xT_s[d][mh][:, mo * 128:(mo + 1) * 128],
                            rhs=wv_s[d][:, :],
                            start=(d == 0),
                            stop=(d == NDT - 1),
                        )
                for k in range(4):
                    m = 4 * r + k
                    va = v_aug[m].rearrange("p (h t) -> p h t", t=DH + 1)
                    nc.vector.tensor_copy(
                        va[:, :, 0:DH],
                        pvt[:, k * 512:(k + 1) * 512].rearrange(
                            "p (h t) -> p h t", t=DH),
                    )
                    nc.vector.memset(va[:, :, DH:DH + 1], 1.0)

            def v_piece(m):
                """4-matmul filler piece: V projection for j-chunk m."""
                pa = psA.tile([128, 1024], f32, name="pa", tag="pa")
                mh, mo = divmod(m, 8)
                for d in range(NDT):
                    nc.tensor.matmul(
                        pa[:, 0:512],
                        lhsT=xT_s[d][mh][:, mo * 128:(mo + 1) * 128],
                        rhs=wv_s[d][:, :],
                        start=(d == 0),
                        stop=(d == NDT - 1),
                    )
                va = v_aug[m].rearrange("p (h t) -> p h t", t=DH + 1)
                nc.vector.tensor_copy(
                    va[:, :, 0:DH],
                    pa[:, 0:512].rearrange("p (h t) -> p h t", t=DH),
                )
                nc.vector.memset(va[:, :, DH:DH + 1], 1.0)

            # ---- lead-in compute: Q/K for heads 0-1, V chunks 0-7 ----
            # qk halves alternate with v0 chunks so the PE never idles on a
            # PSUM-copy wait (gaps reset the clock ramp to mid pstate).
            def qk_half(w_s, dst, ch, half):
                pa = psA.tile([128, 1024], f32, name="pa", tag="pa")
                for d in range(NDT):
                    for nn in range(2):
                        s0 = nn * 512
                        nc.tensor.matmul(
                            pa[:, nn * 512:(nn + 1) * 512],
                            lhsT=w_s[d][:, ch * 128:(ch + 1) * 128],
                            rhs=xT_s[d][half][:, s0:s0 + 512],
                            start=(d == 0),
                            stop=(d == NDT - 1),
                        )
                nc.vector.tensor_copy(
                    dst[ch][:, half * 1024:(half + 1) * 1024], pa[:, :])

            pvt0 = psV.tile([128, 4 * 512], f32, name="pvt", tag="pv")

            def v_chunk_into(k, m):
                for d in range(NDT):
                    mh, mo = divmod(m, 8)
                    nc.tensor.matmul(
                        pvt0[:, k * 512:(k + 1) * 512],
                        lhsT=xT_s[d][mh][:, mo * 128:(mo + 1) * 128],
                        rhs=wv_s[d][:, :],
                        start=(d == 0),
                        stop=(d == NDT - 1),
                    )
                va = v_aug[m].rearrange("p (h t) -> p h t", t=DH + 1)
                nc.vector.tensor_copy(
                    va[:, :, 0:DH],
                    pvt0[:, k * 512:(k + 1) * 512].rearrange(
                        "p (h t) -> p h t", t=DH),
                )
                nc.vector.memset(va[:, :, DH:DH + 1], 1.0)

            qk_half(wq_s, qt_lo, 0, 0)
            v_chunk_into(0, 0)
            qk_half(wq_s, qt_lo, 0, 1)
            v_chunk_into(1, 1)
            swap_half(qt_lo, qt_hi, 0, 0)
            swap_half(qt_lo, qt_hi, 0, 1)
            qk_half(wk_s, kt_lo, 0, 0)
            v_chunk_into(2, 2)
            qk_half(wk_s, kt_lo, 0, 1)
            v_chunk_into(3, 3)
            swap_half(kt_lo, kt_hi, 0, 0)
            swap_half(kt_lo, kt_hi, 0, 1)
            v_round(1)

            # Filler schedule: {head: {jc: [thunk, ...]}}
            fillers = {h: {} for h in range(H)}

            def add_filler(h, jc, fn):
                fillers[h].setdefault(jc, []).append(fn)

            # head 0: V chunks 8..15 on even jcs. Deadline: chunk c is
            # consumed by PV(c) at loop-jc c+1; every piece lands well ahead.
            for i, m in enumerate(range(8, 16)):
                add_filler(0, 2 * i, (lambda m=m: v_piece(m)))
            # heads 1..3: Q/K projection for head pair t=h, plus swaps
            for h in range(1, 4):
                t = h  # chunk index produced during head h
                for i in range(4):
                    half, nn = divmod(i, 2)
                    add_filler(h, 2 * i,
                               (lambda t=t, half=half, nn=nn:
                                qk_piece(wq_s, qt_lo, t, half, nn)))
                add_filler(h, 7, (lambda t=t: swap_half(qt_lo, qt_hi, t, 0)))
                add_filler(h, 9, (lambda t=t: swap_half(qt_lo, qt_hi, t, 1)))
                for i in range(4):
                    half, nn = divmod(i, 2)
                    add_filler(h, 8 + 2 * i,
                               (lambda t=t, half=half, nn=nn:
                                qk_piece(wk_s, kt_lo, t, half, nn)))
                add_filler(h, 11, (lambda t=t: swap_half(kt_lo, kt_hi, t, 0)))
                add_filler(h, 15, (lambda t=t: swap_half(kt_lo, kt_hi, t, 1)))

            # ---- attention, head by head ----
            # PE order per j-chunk: scores(jc) then PV(jc-1), so the PE FIFO
            # never waits on exp(jc) before issuing scores(jc+1).
            ot = [apool.tile([128, S], bf16, name=f"ot{t}", tag=f"ot{t}")
                  for t in range(NDT)]
            recip_row = wpool.tile([1, S], f32, name="rrow", tag="rrow")
            pv_last = None
            for h in range(H):
                t, p = h // 2, h % 2
                lo_sl = slice(64 * p, 64 * p + 64)
                hi_sl = slice(64 * (1 - p), 64 * (1 - p) + 64)
                pv = psV.tile([128, 4 * 512], f32, name="pvh", tag="pv")
                ets = {}

                def pv_mms(jc):
                    for it in range(NST):
                        nc.tensor.matmul(
                            pv[0:DH + 1, it * 512:(it + 1) * 512],
                            lhsT=v_aug[jc][:, h * (DH + 1):(h + 1) * (DH + 1)],
                            rhs=ets[jc][:, it * 512:(it + 1) * 512],
                            start=(jc == 0),
                            stop=(jc == NSC - 1),
                        )

                trail = 1
                for jc in range(NSC):
                    et = epool.tile([128, S], bf16, name="et", tag="et")
                    ets[jc] = et
                    for half in range(2):
                        pa = psA.tile([128, 1024], f32, name="pa", tag="pa")
                        i0, i1 = 2 * half, 2 * half + 1
                        nc.tensor.matmul(
                            pa[:, 0:512],
                            lhsT=kt_lo[t][lo_sl, jc * 128:(jc + 1) * 128],
                            rhs=qt_lo[t][lo_sl, i0 * 512:(i0 + 1) * 512],
                        )
                        nc.tensor.matmul(
                            pa[:, 512:1024],
                            lhsT=kt_hi[t][hi_sl, jc * 128:(jc + 1) * 128],
                            rhs=qt_hi[t][hi_sl, i1 * 512:(i1 + 1) * 512],
                        )
                        nc.scalar.activation(
                            out=et[:, half * 1024:(half + 1) * 1024],
                            in_=pa[:, :],
                            func=Exp,
                            scale=SCALE,
                        )
                    # Fillers go between the score pair and PV: the filler's
                    # PSUM buffer then drains well before the scores two
                    # rotations later need it, so the exp stream never waits.
                    for fn in fillers[h].get(jc, ()):
                        fn()
                    if jc >= trail:
                        pv_mms(jc - trail)
                for jc in range(NSC - trail, NSC):
                    pv_mms(jc)

                if h < H - 1:
                    # Decouple normalization from the PE pipeline: get O_un
                    # and the denominator row out of PSUM fast, then
                    # normalize via a (128,16) reciprocal + DRAM
                    # partition-broadcast. Fully hidden under head h+1.
                    oun = spool.tile([DH + 1, S], f32, name="oun", tag="oun")
                    nc.vector.tensor_copy(oun[:, :], pv[0:DH + 1, :])
                    den128 = spool.tile([128, 16], f32, name="den128",
                                        tag="d128")
                    nc.sync.dma_start(out=den128[:, :], in_=oun[DH:DH + 1, :])
                    nc.vector.reciprocal(out=den128[:, :], in_=den128[:, :])
                    nc.sync.dma_start(out=den_dram[h, :], in_=den128[:, :])
                    bc = spool.tile([64, S], f32, name="bc", tag="bc")
                    dd = den_dram[h:h + 1, :]
                    bcast_src = bass.AP(
                        tensor=dd.tensor,
                        offset=dd.offset,
                        ap=[[0, 64]] + [list(x) for x in dd.ap[1:]],
                    )
                    nc.sync.dma_start(out=bc[:, :], in_=bcast_src)
                    nc.vector.tensor_mul(
                        ot[t][64 * p:64 * p + 64, :], oun[0:DH, :], bc[:, :])
                else:
                    pv_last = pv

            # ---- head 7 fast-path normalization (critical tail) ----
            # reciprocal of the PSUM denominator row via a (128,16) reshape
            # round trip (small SBUF-local DMAs), GpSimd partition-broadcast,
            # multiply straight out of PSUM.
            bc7 = spool.tile([64, S], f32, name="bc", tag="bc")
            for hh in range(2):
                sl = slice(hh * 1024, (hh + 1) * 1024)
                nc.vector.tensor_copy(recip_row[0:1, sl],
                                      pv_last[DH:DH + 1, sl])
                den7 = spool.tile([128, 8], f32, name="den7", tag="d7")
                nc.sync.dma_start(out=den7[:, :], in_=recip_row[0:1, sl])
                nc.vector.reciprocal(out=den7[:, :], in_=den7[:, :])
                nc.sync.dma_start(out=recip_row[0:1, sl], in_=den7[:, :])
                nc.gpsimd.partition_broadcast(bc7[:, sl], recip_row[0:1, sl],
                                              channels=64)
                nc.vector.tensor_mul(
                    ot[3][64:128, sl], pv_last[0:DH, sl], bc7[:, sl])
            # keep the PE clock hot through the normalization bubble
            for k in range(14):
                ja = psA.tile([128, 1024], f32, name="pa", tag="pa")
                for half in range(2):
                    nc.tensor.matmul(
                        ja[:, half * 512:(half + 1) * 512],
                        lhsT=junk_sb[:, 0:128],
                        rhs=junk_sb[:, :],
                    )

            # ---- output projection (psA ping-pong so matmul groups and the
            # bias-add/copy of the previous group overlap) ----
            for ch in range(NDT):
                stage = opool.tile([128, S], f32, name="stage", tag="stage")
                for half in range(2):
                    po = psA.tile([128, 1024], f32, name="pa", tag="pa")
                    for st2 in range(2):
                        st = half * 2 + st2
                        for kt in range(NDT):
                            nc.tensor.matmul(
                                po[:, st2 * 512:(st2 + 1) * 512],
                                lhsT=wo_s[kt][:, ch * 128:(ch + 1) * 128],
                                rhs=ot[kt][:, st * 512:(st + 1) * 512],
                                start=(kt == 0),
                                stop=(kt == NDT - 1),
                            )
                    nc.vector.tensor_scalar_add(
                        out=stage[:, half * 1024:(half + 1) * 1024],
                        in0=po[:, :],
                        scalar1=bo_s[ch][:, :],
                    )
                    nc.sync.dma_start(
                        out=out[ch * 128:(ch + 1) * 128,
                                half * 1024:(half + 1) * 1024],
                        in_=stage[:, half * 1024:(half + 1) * 1024],
                    )

    nc.finalize()
    return nc


_NC_CACHE = None


def _get_nc():
    global _NC_CACHE
    if _NC_CACHE is None:
        _NC_CACHE = _build_kernel()
    return _NC_CACHE


def kernel(x, W_qkv, W_out, b_out):
    from concourse.bass_utils import run_bass_kernel_spmd

    bf16 = ml_dtypes.bfloat16

    # head-interleave and transpose the qkv weight: row 192h+{0,64,128}+c of
    # W_qkv is q/k/v row (h, c); regroup to e' = 64h+c and transpose to [d, e']
    w3 = W_qkv.reshape(H, 3, DH, D)
    wq_h = np.ascontiguousarray(w3[:, 0].reshape(INNER, D).T).astype(bf16)
    wk_h = np.ascontiguousarray(w3[:, 1].reshape(INNER, D).T).astype(bf16)
    wv_h = np.ascontiguousarray(w3[:, 2].reshape(INNER, D).T).astype(bf16)
    wo_h = np.ascontiguousarray(W_out.T).astype(bf16)  # [hc, d]
    bo_h = np.ascontiguousarray(b_out.reshape(NDT, 128, 1)).astype(np.float32)

    in_maps = []
    for b in range(N_CORES):
        xT_b = np.ascontiguousarray(x[b].T).astype(bf16)  # [d, s]
        in_maps.append({
            "xT": xT_b, "wq": wq_h, "wk": wk_h, "wv": wv_h,
            "wo": wo_h, "bo": bo_h,
        })

    nc = _get_nc()
    res = run_bass_kernel_spmd(nc, in_maps, list(range(N_CORES)))
    outs = [res.results[b]["out"].T for b in range(N_CORES)]  # [s, d] each
    return np.ascontiguousarray(np.stack(outs, axis=0)).astype(np.float32)


# revision 13
# speedup vs baseline: 1.1843x; 1.1843x over previous
"""Multi-head attention (B=8, S=2048, D=512, H=8, DH=64) on 8 TRN2 NeuronCores.

Strategy: data-parallel over the batch dim — core b computes batch element b
end-to-end (no collectives). Per core, everything is kept transposed
("feature on partitions") so that softmax reductions land on the TensorE
contraction axis. The kernel is ACT-bound (exp over S^2 per head), so the
schedule keeps the Activation engine saturated from ~25us onward:

  1. Lead-in: load x/wq/wk, junk-warm the PE, project Q/K for head pair 0
     and V chunks 0..7, then enter the head loop immediately.
  2. Remaining QKV projection work (V chunks 8..15, Q/K for head pairs
     1..3) is injected into the attention loop as small 4-matmul "filler
     pieces" on alternating j-chunks, sized to fit the PE slack under the
     exp stream (ACT: ~2390ns/jc, PE: scores pair 518 + PV 1036).
  3. Scores are computed transposed via row-packed K=64 matmul pairs into
     disjoint PE row groups; exp(scale*S^T) runs on ScalarE out of PSUM.
  4. O^T = Vaug E^T accumulates over j with a ones-row denominator.
  5. Heads 0-6 normalize via reciprocal + DRAM partition-broadcast DMA
     (fully hidden under the next head's exp stream). Head 7 (critical
     tail) normalizes on-chip: DVE reciprocal of the PSUM denominator row,
     GpSimd partition_broadcast, DVE multiply straight out of PSUM; junk
     matmuls keep the PE hot so the output projection runs at full clock.
"""

import numpy as np
import ml_dtypes

B, S, D = 8, 2048, 512
H, DH = 8, 64
INNER = H * DH
SCALE = DH ** -0.5

N_CORES = 8
NDT = D // 128   # 4 contraction tiles
NSC = S // 128   # 16 s-chunks (j-chunks)
NST = S // 512   # 4 s-tiles


def _build_kernel():
    import concourse.bass as bass
    import concourse.mybir as mybir
    import concourse.tile as tile
    from concourse import bacc

    bf16 = mybir.dt.bfloat16
    f32 = mybir.dt.float32
    Exp = mybir.ActivationFunctionType.Exp

    nc = bacc.Bacc()

    xT = nc.declare_dram_parameter("xT", [D, S], bf16, isOutput=False)
    wq = nc.declare_dram_parameter("wq", [D, INNER], bf16, isOutput=False)
    wk = nc.declare_dram_parameter("wk", [D, INNER], bf16, isOutput=False)
    wv = nc.declare_dram_parameter("wv", [D, INNER], bf16, isOutput=False)
    wo = nc.declare_dram_parameter("wo", [INNER, D], bf16, isOutput=False)
    bo = nc.declare_dram_parameter("bo", [NDT, 128, 1], f32, isOutput=False)
    out = nc.declare_dram_parameter("out", [D, S], f32, isOutput=True)
    den_dram = nc.dram_tensor("den_scratch", [H, S], f32)

    with tile.TileContext(nc) as tc:
        with (
            tc.tile_pool(name="weights", bufs=1) as wpool,
            tc.tile_pool(name="acts", bufs=1) as apool,
            tc.tile_pool(name="et", bufs=4) as epool,
            tc.tile_pool(name="small", bufs=2) as spool,
            tc.tile_pool(name="ostage", bufs=2) as opool,
            tc.tile_pool(name="psA", bufs=2, space="PSUM") as psA,
            tc.tile_pool(name="psV", bufs=1, space="PSUM") as psV,
        ):
            # ---- load inputs (x and q/k weights first: they gate head 0) ----
            xT_s = [[wpool.tile([128, S // 2], bf16, name=f"xT{d}_{hf}",
                              tag=f"xT{d}_{hf}") for hf in range(2)]
                    for d in range(NDT)]
            wq_s = [wpool.tile([128, INNER], bf16, name=f"wq{d}", tag=f"wq{d}")
                    for d in range(NDT)]
            wk_s = [wpool.tile([128, INNER], bf16, name=f"wk{d}", tag=f"wk{d}")
                    for d in range(NDT)]
            wv_s = [wpool.tile([128, INNER], bf16, name=f"wv{d}", tag=f"wv{d}")
                    for d in range(NDT)]
            wo_s = [wpool.tile([128, D], bf16, name=f"wo{d}", tag=f"wo{d}")
                    for d in range(NDT)]
            bo_s = [wpool.tile([128, 1], f32, name=f"bo{d}", tag=f"bo{d}")
                    for d in range(NDT)]
            for d in range(NDT):
                sl = slice(d * 128, (d + 1) * 128)
                nc.sync.dma_start(out=xT_s[d][0][:], in_=xT[sl, 0:S // 2])
                nc.sync.dma_start(out=wq_s[d][:], in_=wq[sl, :])
                nc.scalar.dma_start(out=xT_s[d][1][:], in_=xT[sl, S // 2:])
                nc.scalar.dma_start(out=wk_s[d][:], in_=wk[sl, :])
                nc.gpsimd.dma_start(out=wv_s[d][:], in_=wv[sl, :])
            for d in range(NDT):
                sl = slice(d * 128, (d + 1) * 128)
                nc.gpsimd.dma_start(out=wo_s[d][:], in_=wo[sl, :])
                nc.gpsimd.dma_start(out=bo_s[d][:], in_=bo[d, :, :])

            # ---- QKV target tiles ----
            qt_lo = [apool.tile([128, S], bf16, name=f"qlo{t}", tag=f"qlo{t}")
                     for t in range(NDT)]
            kt_lo = [apool.tile([128, S], bf16, name=f"klo{t}", tag=f"klo{t}")
                     for t in range(NDT)]
            qt_hi = [apool.tile([128, S], bf16, name=f"qhi{t}", tag=f"qhi{t}")
                     for t in range(NDT)]
            kt_hi = [apool.tile([128, S], bf16, name=f"khi{t}", tag=f"khi{t}")
                     for t in range(NDT)]
            v_aug = [apool.tile([128, H * (DH + 1)], bf16, name=f"va{m}",
                                tag=f"va{m}") for m in range(NSC)]

            # PE warm-up: junk matmuls during the input-DMA window keep the
            # HAM activity monitor busy so real matmuls start fast.
            junk_sb = wpool.tile([128, 512], bf16, name="junk", tag="junk")
            nc.vector.memset(junk_sb[:, :], 0.0)
            # Trigger the GpSimd ucode library load now (it is lazy); the
            # head-7 partition_broadcast would otherwise eat a ~7us
            # LOAD_LIB on the critical tail.
            pre_bc = wpool.tile([2, 16], f32, name="prebc", tag="prebc")
            nc.vector.memset(pre_bc[:, :], 1.0)
            nc.gpsimd.partition_broadcast(pre_bc[:, :], pre_bc[0:1, :],
                                          channels=2)
            junk_ps = psV.tile([128, 4 * 512], f32, name="junkps", tag="pv")
            for k in range(6):
                nc.tensor.matmul(
                    junk_ps[:, (k % 4) * 512:(k % 4 + 1) * 512],
                    lhsT=junk_sb[:, 0:128],
                    rhs=junk_sb[:, :],
                )

            def qk_chunk(w_s, dst, ch, eng):
                """Full Q or K projection for head pair ch (lead-in only)."""
                for half in range(2):
                    pa = psA.tile([128, 1024], f32, name="pa", tag="pa")
                    for d in range(NDT):
                        for nn in range(2):
                            s0 = nn * 512
                            nc.tensor.matmul(
                                pa[:, nn * 512:(nn + 1) * 512],
                                lhsT=w_s[d][:, ch * 128:(ch + 1) * 128],
                                rhs=xT_s[d][half][:, s0:s0 + 512],
                                start=(d == 0),
                                stop=(d == NDT - 1),
                            )
                    eng.tensor_copy(
                        dst[ch][:, half * 1024:(half + 1) * 1024], pa[:, :])

            def qk_piece(w_s, dst, ch, half, nn):
                """4-matmul filler piece: one 512-col block of a projection."""
                pa = psA.tile([128, 1024], f32, name="pa", tag="pa")
                s0 = nn * 512
                for d in range(NDT):
                    nc.tensor.matmul(
                        pa[:, 0:512],
                        lhsT=w_s[d][:, ch * 128:(ch + 1) * 128],
                        rhs=xT_s[d][half][:, s0:s0 + 512],
                        start=(d == 0),
                        stop=(d == NDT - 1),
                    )
                off = half * 1024 + nn * 512
                nc.vector.tensor_copy(dst[ch][:, off:off + 512], pa[:, 0:512])

            def swap_half(src_lo, src_hi, t, sh):
                """DMA-swap one 1024-wide s-half of the lo replica into hi."""
                s0, s1 = sh * 1024, (sh + 1) * 1024
                nc.sync.dma_start(out=src_hi[t][64:128, s0:s1],
                                  in_=src_lo[t][0:64, s0:s1])
                nc.sync.dma_start(out=src_hi[t][0:64, s0:s1],
                                  in_=src_lo[t][64:128, s0:s1])

            def v_round(r):
                """V projection for j-chunks 4r..4r+3 (lead-in only)."""
                pvt = psV.tile([128, 4 * 512], f32, name="pvt", tag="pv")
                for k in range(4):
                    m = 4 * r + k
                    for d in range(NDT):
                        mh, mo = divmod(m, 8)
                        nc.tensor.matmul(
                            pvt[:, k * 512:(k + 1) * 512],
                            lhsT=xT_s[d][mh][:, mo * 128:(mo + 1) * 128],
                            rhs=wv_s[d][:, :],
                            start=(d == 0),
                            stop=(d == NDT - 1),
                        )
                for k in range(4):
                    m = 4 * r + k
                    va = v_aug[m].rearrange("p (h t) -> p h t", t=DH + 1)
                    nc.vector.tensor_copy(
                        va[:, :, 0:DH],
                        pvt[:, k * 512:(k + 1) * 512].rearrange(
                            "p (h t) -> p h t", t=DH),
                    )
                    nc.vector.memset(va[:, :, DH:DH + 1], 1.0)

            def v_piece(m):
                """4-matmul filler piece: V projection for j-chunk m."""
                pa = psA.tile([128, 1024], f32, name="pa", tag="pa")
                mh, mo = divmod(m, 8)
                for d in range(NDT):
                    nc.tensor.matmul(
                        pa[:, 0:512],
                        lhsT=xT_s[d][mh][:, mo * 128:(mo + 1) * 128],
                        rhs=wv_s[d][:, :],
                        start=(d == 0),
                        stop=(d == NDT - 1),
                    )
                va = v_aug[m].rearrange("p (h t) -> p h t", t=DH + 1)
                nc.vector.tensor_copy(
                    va[:, :, 0:DH],
                    pa[:, 0:512].rearrange("p (h t) -> p h t", t=DH),
                )
                nc.vector.memset(va[:, :, DH:DH + 1], 1.0)

            # ---- lead-in compute: Q/K for heads 0-1, V chunks 0-7 ----
            # qk halves alternate with v0 chunks so the PE never idles on a
            # PSUM-copy wait (gaps reset the clock ramp to mid pstate).
            def qk_half(w_s, dst, ch, half):
                pa = psA.tile([128, 1024], f32, name="pa", tag="pa")
                for d in range(NDT):
                    for nn in range(2):
                        s0 = nn * 512
                        nc.tensor.matmul(
                            pa[:, nn * 512:(nn + 1) * 512],
                            lhsT=w_s[d][:, ch * 128:(ch + 1) * 128],
                            rhs=xT_s[d][half][:, s0:s0 + 512],
                            start=(d == 0),
                            stop=(d == NDT - 1),
                        )
                nc.vector.tensor_copy(
                    dst[ch][:, half * 1024:(half + 1) * 1024], pa[:, :])

            pvt0 = psV.tile([128, 4 * 512], f32, name="pvt", tag="pv")

            def v_chunk_into(k, m):
                for d in range(NDT):
                    mh, mo = divmod(m, 8)
                    nc.tensor.matmul(
                        pvt0[:, k * 512:(k + 1) * 512],
                        lhsT=xT_s[d][mh][:, mo * 128:(mo + 1) * 128],
                        rhs=wv_s[d][:, :],
                        start=(d == 0),
                        stop=(d == NDT - 1),
                    )
                va = v_aug[m].rearrange("p (h t) -> p h t", t=DH + 1)
                nc.vector.tensor_copy(
                    va[:, :, 0:DH],
                    pvt0[:, k * 512:(k + 1) * 512].rearrange(
                        "p (h t) -> p h t", t=DH),
                )
                nc.vector.memset(va[:, :, DH:DH + 1], 1.0)

            qk_half(wq_s, qt_lo, 0, 0)
            v_chunk_into(0, 0)
            qk_half(wq_s, qt_lo, 0, 1)
            v_chunk_into(1, 1)
            swap_half(qt_lo, qt_hi, 0, 0)
            swap_half(qt_lo, qt_hi, 0, 1)
            qk_half(wk_s, kt_lo, 0, 0)
            v_chunk_into(2, 2)
            qk_half(wk_s, kt_lo, 0, 1)
            v_chunk_into(3, 3)
            swap_half(kt_lo, kt_hi, 0, 0)
            swap_half(kt_lo, kt_hi, 0, 1)
            v_round(1)

            # Filler schedule: {head: {jc: [thunk, ...]}}
            fillers = {h: {} for h in range(H)}

            def add_filler(h, jc, fn):
                fillers[h].setdefault(jc, []).append(fn)

            # head 0: V chunks 8..15 on even jcs. Deadline: chunk c is
            # consumed by PV(c) at loop-jc c+1; every piece lands well ahead.
            for i, m in enumerate(range(8, 16)):
                add_filler(0, 2 * i, (lambda m=m: v_piece(m)))
            # heads 1..3: Q/K projection for head pair t=h, plus swaps
            for h in range(1, 4):
                t = h  # chunk index produced during head h
                for i in range(4):
                    half, nn = divmod(i, 2)
                    add_filler(h, 2 * i,
                               (lambda t=t, half=half, nn=nn:
                                qk_piece(wq_s, qt_lo, t, half, nn)))
                add_filler(h, 7, (lambda t=t: swap_half(qt_lo, qt_hi, t, 0)))
                add_filler(h, 9, (lambda t=t: swap_half(qt_lo, qt_hi, t, 1)))
                for i in range(4):
                    half, nn = divmod(i, 2)
                    add_filler(h, 8 + 2 * i,
                               (lambda t=t, half=half, nn=nn:
                                qk_piece(wk_s, kt_lo, t, half, nn)))
                add_filler(h, 11, (lambda t=t: swap_half(kt_lo, kt_hi, t, 0)))
                add_filler(h, 15, (lambda t=t: swap_half(kt_lo, kt_hi, t, 1)))

            # ---- attention, head by head ----
            # PE order per j-chunk: scores(jc) then PV(jc-1), so the PE FIFO
            # never waits on exp(jc) before issuing scores(jc+1).
            ot = [apool.tile([128, S], bf16, name=f"ot{t}", tag=f"ot{t}")
                  for t in range(NDT)]
            recip_row = wpool.tile([1, S], f32, name="rrow", tag="rrow")
            pv_last = None
            for h in range(H):
                t, p = h // 2, h % 2
                lo_sl = slice(64 * p, 64 * p + 64)
                hi_sl = slice(64 * (1 - p), 64 * (1 - p) + 64)
                pv = psV.tile([128, 4 * 512], f32, name="pvh", tag="pv")
                ets = {}

                def pv_mms(jc):
                    for it in range(NST):
                        nc.tensor.matmul(
                            pv[0:DH + 1, it * 512:(it + 1) * 512],
                            lhsT=v_aug[jc][:, h * (DH + 1):(h + 1) * (DH + 1)],
                            rhs=ets[jc][:, it * 512:(it + 1) * 512],
                            start=(jc == 0),
                            stop=(jc == NSC - 1),
                        )

                trail = 1
                for jc in range(NSC):
                    et = epool.tile([128, S], bf16, name="et", tag="et")
                    ets[jc] = et
                    for half in range(2):
                        pa = psA.tile([128, 1024], f32, name="pa", tag="pa")
                        i0, i1 = 2 * half, 2 * half + 1
                        nc.tensor.matmul(
                            pa[:, 0:512],
                            lhsT=kt_lo[t][lo_sl, jc * 128:(jc + 1) * 128],
                            rhs=qt_lo[t][lo_sl, i0 * 512:(i0 + 1) * 512],
                        )
                        nc.tensor.matmul(
                            pa[:, 512:1024],
                            lhsT=kt_hi[t][hi_sl, jc * 128:(jc + 1) * 128],
                            rhs=qt_hi[t][hi_sl, i1 * 512:(i1 + 1) * 512],
                        )
                        nc.scalar.activation(
                            out=et[:, half * 1024:(half + 1) * 1024],
                            in_=pa[:, :],
                            func=Exp,
                            scale=SCALE,
                        )
                    # Fillers go between the score pair and PV: the filler's
                    # PSUM buffer then drains well before the scores two
                    # rotations later need it, so the exp stream never waits.
                    for fn in fillers[h].get(jc, ()):
                        fn()
                    if jc >= trail:
                        pv_mms(jc - trail)
                for jc in range(NSC - trail, NSC):
                    pv_mms(jc)

                if h < H - 1:
                    # Decouple normalization from the PE pipeline: get O_un
                    # and the denominator row out of PSUM fast, then
                    # normalize via a (128,16) reciprocal + DRAM
                    # partition-broadcast. Fully hidden under head h+1.
                    oun = spool.tile([DH + 1, S], f32, name="oun", tag="oun")
                    nc.vector.tensor_copy(oun[:, :], pv[0:DH + 1, :])
                    den128 = spool.tile([128, 16], f32, name="den128",
                                        tag="d128")
                    nc.sync.dma_start(out=den128[:, :], in_=oun[DH:DH + 1, :])
                    nc.vector.reciprocal(out=den128[:, :], in_=den128[:, :])
                    nc.sync.dma_start(out=den_dram[h, :], in_=den128[:, :])
                    bc = spool.tile([64, S], f32, name="bc", tag="bc")
                    dd = den_dram[h:h + 1, :]
                    bcast_src = bass.AP(
                        tensor=dd.tensor,
                        offset=dd.offset,
                        ap=[[0, 64]] + [list(x) for x in dd.ap[1:]],
                    )
                    nc.sync.dma_start(out=bc[:, :], in_=bcast_src)
                    nc.vector.tensor_mul(
                        ot[t][64 * p:64 * p + 64, :], oun[0:DH, :], bc[:, :])
                else:
                    pv_last = pv

            # ---- head 7 fast-path normalization (critical tail) ----
            # reciprocal of the PSUM denominator row via a (128,16) reshape
            # round trip (small SBUF-local DMAs), GpSimd partition-broadcast,
            # multiply straight out of PSUM.
            nc.vector.tensor_copy(recip_row[0:1, :], pv_last[DH:DH + 1, :])
            den7 = spool.tile([128, 16], f32, name="den128", tag="d128")
            nc.sync.dma_start(out=den7[:, :], in_=recip_row[0:1, :])
            nc.vector.reciprocal(out=den7[:, :], in_=den7[:, :])
            nc.sync.dma_start(out=recip_row[0:1, :], in_=den7[:, :])
            bc7 = spool.tile([64, S], f32, name="bc", tag="bc")
            nc.gpsimd.partition_broadcast(bc7[:, :], recip_row[0:1, :],
                                          channels=64)
            # keep the PE clock hot through the normalization bubble
            for k in range(14):
                ja = psA.tile([128, 1024], f32, name="pa", tag="pa")
                for half in range(2):
                    nc.tensor.matmul(
                        ja[:, half * 512:(half + 1) * 512],
                        lhsT=junk_sb[:, 0:128],
                        rhs=junk_sb[:, :],
                    )
            nc.vector.tensor_mul(
                ot[3][64:128, :], pv_last[0:DH, :], bc7[:, :])

            # ---- output projection (psA ping-pong so matmul groups and the
            # bias-add/copy of the previous group overlap) ----
            for ch in range(NDT):
                stage = opool.tile([128, S], f32, name="stage", tag="stage")
                for half in range(2):
                    po = psA.tile([128, 1024], f32, name="pa", tag="pa")
                    for st2 in range(2):
                        st = half * 2 + st2
                        for kt in range(NDT):
                            nc.tensor.matmul(
                                po[:, st2 * 512:(st2 + 1) * 512],
                                lhsT=wo_s[kt][:, ch * 128:(ch + 1) * 128],
                                rhs=ot[kt][:, st * 512:(st + 1) * 512],
                                start=(kt == 0),
                                stop=(kt == NDT - 1),
                            )
                    nc.vector.tensor_scalar_add(
                        out=stage[:, half * 1024:(half + 1) * 1024],
                        in0=po[:, :],
                        scalar1=bo_s[ch][:, :],
                    )
                    nc.sync.dma_start(
                        out=out[ch * 128:(ch + 1) * 128,
                                half * 1024:(half + 1) * 1024],
                        in_=stage[:, half * 1024:(half + 1) * 1024],
                    )

    nc.finalize()
    return nc


_NC_CACHE = None


def _get_nc():
    global _NC_CACHE
    if _NC_CACHE is None:
        _NC_CACHE = _build_kernel()
    return _NC_CACHE


def kernel(x, W_qkv, W_out, b_out):
    from concourse.bass_utils import run_bass_kernel_spmd

    bf16 = ml_dtypes.bfloat16

    # head-interleave and transpose the qkv weight: row 192h+{0,64,128}+c of
    # W_qkv is q/k/v row (h, c); regroup to e' = 64h+c and transpose to [d, e']
    w3 = W_qkv.reshape(H, 3, DH, D)
    wq_h = np.ascontiguousarray(w3[:, 0].reshape(INNER, D).T).astype(bf16)
    wk_h = np.ascontiguousarray(w3[:, 1].reshape(INNER, D).T).astype(bf16)
    wv_h = np.ascontiguousarray(w3[:, 2].reshape(INNER, D).T).astype(bf16)
    wo_h = np.ascontiguousarray(W_out.T).astype(bf16)  # [hc, d]
    bo_h = np.ascontiguousarray(b_out.reshape(NDT, 128, 1)).astype(np.float32)

    in_maps = []
    for b in range(N_CORES):
        xT_b = np.ascontiguousarray(x[b].T).astype(bf16)  # [d, s]
        in_maps.append({
            "xT": xT_b, "wq": wq_h, "wk": wk_h, "wv": wv_h,
            "wo": wo_h, "bo": bo_h,
        })

    nc = _get_nc()
    res = run_bass_kernel_spmd(nc, in_maps, list(range(N_CORES)))
    outs = [res.results[b]["out"].T for b in range(N_CORES)]  # [s, d] each
    return np.ascontiguousarray(np.stack(outs, axis=0)).astype(np.float32)


# revision 15
# speedup vs baseline: 1.2003x; 1.0135x over previous
"""Multi-head attention (B=8, S=2048, D=512, H=8, DH=64) on 8 TRN2 NeuronCores.

Strategy: data-parallel over the batch dim — core b computes batch element b
end-to-end (no collectives). Per core, everything is kept transposed
("feature on partitions") so that softmax reductions land on the TensorE
contraction axis. The kernel is ACT-bound (exp over S^2 per head), so the
schedule keeps the Activation engine saturated from ~25us onward:

  1. Lead-in: load x/wq/wk, junk-warm the PE, project Q/K for head pair 0
     and V chunks 0..7, then enter the head loop immediately.
  2. Remaining QKV projection work (V chunks 8..15, Q/K for head pairs
     1..3) is injected into the attention loop as small 4-matmul "filler
     pieces" on alternating j-chunks, sized to fit the PE slack under the
     exp stream (ACT: ~2390ns/jc, PE: scores pair 518 + PV 1036).
  3. Scores are computed transposed via row-packed K=64 matmul pairs into
     disjoint PE row groups; exp(scale*S^T) runs on ScalarE out of PSUM.
  4. O^T = Vaug E^T accumulates over j with a ones-row denominator.
  5. Heads 0-6 normalize via reciprocal + DRAM partition-broadcast DMA
     (fully hidden under the next head's exp stream). Head 7 (critical
     tail) normalizes on-chip: DVE reciprocal of the PSUM denominator row,
     GpSimd partition_broadcast, DVE multiply straight out of PSUM; junk
     matmuls keep the PE hot so the output projection runs at full clock.
"""

import numpy as np
import ml_dtypes

B, S, D = 8, 2048, 512
H, DH = 8, 64
INNER = H * DH
SCALE = DH ** -0.5

N_CORES = 8
NDT = D // 128   # 4 contraction tiles
NSC = S // 128   # 16 s-chunks (j-chunks)
NST = S // 512   # 4 s-tiles


def _build_kernel():
    import concourse.bass as bass
    import concourse.mybir as mybir
    import concourse.tile as tile
    from concourse import bacc

    bf16 = mybir.dt.bfloat16
    f32 = mybir.dt.float32
    Exp = mybir.ActivationFunctionType.Exp

    nc = bacc.Bacc()

    xT = nc.declare_dram_parameter("xT", [D, S], bf16, isOutput=False)
    wq = nc.declare_dram_parameter("wq", [D, INNER], bf16, isOutput=False)
    wk = nc.declare_dram_parameter("wk", [D, INNER], bf16, isOutput=False)
    wv = nc.declare_dram_parameter("wv", [D, INNER], bf16, isOutput=False)
    wo = nc.declare_dram_parameter("wo", [INNER, D], bf16, isOutput=False)
    bo = nc.declare_dram_parameter("bo", [NDT, 128, 1], f32, isOutput=False)
    out = nc.declare_dram_parameter("out", [D, S], f32, isOutput=True)
    den_dram = nc.dram_tensor("den_scratch", [H, S], f32)

    with tile.TileContext(nc) as tc:
        with (
            tc.tile_pool(name="weights", bufs=1) as wpool,
            tc.tile_pool(name="acts", bufs=1) as apool,
            tc.tile_pool(name="et", bufs=4) as epool,
            tc.tile_pool(name="small", bufs=2) as spool,
            tc.tile_pool(name="ostage", bufs=2) as opool,
            tc.tile_pool(name="psA", bufs=2, space="PSUM") as psA,
            tc.tile_pool(name="psV", bufs=1, space="PSUM") as psV,
        ):
            # ---- load inputs (x and q/k weights first: they gate head 0) ----
            xT_s = [[wpool.tile([128, S // 2], bf16, name=f"xT{d}_{hf}",
                              tag=f"xT{d}_{hf}") for hf in range(2)]
                    for d in range(NDT)]
            wq_s = [wpool.tile([128, INNER], bf16, name=f"wq{d}", tag=f"wq{d}")
                    for d in range(NDT)]
            wk_s = [wpool.tile([128, INNER], bf16, name=f"wk{d}", tag=f"wk{d}")
                    for d in range(NDT)]
            wv_s = [wpool.tile([128, INNER], bf16, name=f"wv{d}", tag=f"wv{d}")
                    for d in range(NDT)]
            wo_s = [wpool.tile([128, D], bf16, name=f"wo{d}", tag=f"wo{d}")
                    for d in range(NDT)]
            bo_s = [wpool.tile([128, 1], f32, name=f"bo{d}", tag=f"bo{d}")
                    for d in range(NDT)]
            for d in range(NDT):
                sl = slice(d * 128, (d + 1) * 128)
                nc.sync.dma_start(out=xT_s[d][0][:], in_=xT[sl, 0:S // 2])
                nc.sync.dma_start(out=wq_s[d][:], in_=wq[sl, :])
                nc.scalar.dma_start(out=xT_s[d][1][:], in_=xT[sl, S // 2:])
                nc.scalar.dma_start(out=wk_s[d][:], in_=wk[sl, :])
                nc.gpsimd.dma_start(out=wv_s[d][:], in_=wv[sl, :])
            for d in range(NDT):
                sl = slice(d * 128, (d + 1) * 128)
                nc.gpsimd.dma_start(out=wo_s[d][:], in_=wo[sl, :])
                nc.gpsimd.dma_start(out=bo_s[d][:], in_=bo[d, :, :])

            # ---- QKV target tiles ----
            qt_lo = [apool.tile([128, S], bf16, name=f"qlo{t}", tag=f"qlo{t}")
                     for t in range(NDT)]
            kt_lo = [apool.tile([128, S], bf16, name=f"klo{t}", tag=f"klo{t}")
                     for t in range(NDT)]
            qt_hi = [apool.tile([128, S], bf16, name=f"qhi{t}", tag=f"qhi{t}")
                     for t in range(NDT)]
            kt_hi = [apool.tile([128, S], bf16, name=f"khi{t}", tag=f"khi{t}")
                     for t in range(NDT)]
            v_aug = [apool.tile([128, H * (DH + 1)], bf16, name=f"va{m}",
                                tag=f"va{m}") for m in range(NSC)]

            # PE warm-up: junk matmuls during the input-DMA window keep the
            # HAM activity monitor busy so real matmuls start fast.
            junk_sb = wpool.tile([128, 512], bf16, name="junk", tag="junk")
            nc.vector.memset(junk_sb[:, :], 0.0)
            # Trigger the GpSimd ucode library load now (it is lazy); the
            # head-7 partition_broadcast would otherwise eat a ~7us
            # LOAD_LIB on the critical tail.
            pre_bc = wpool.tile([2, 16], f32, name="prebc", tag="prebc")
            nc.vector.memset(pre_bc[:, :], 1.0)
            nc.gpsimd.partition_broadcast(pre_bc[:, :], pre_bc[0:1, :],
                                          channels=2)
            junk_ps = psV.tile([128, 4 * 512], f32, name="junkps", tag="pv")
            for k in range(6):
                nc.tensor.matmul(
                    junk_ps[:, (k % 4) * 512:(k % 4 + 1) * 512],
                    lhsT=junk_sb[:, 0:128],
                    rhs=junk_sb[:, :],
                )

            def qk_chunk(w_s, dst, ch, eng):
                """Full Q or K projection for head pair ch (lead-in only)."""
                for half in range(2):
                    pa = psA.tile([128, 1024], f32, name="pa", tag="pa")
                    for d in range(NDT):
                        for nn in range(2):
                            s0 = nn * 512
                            nc.tensor.matmul(
                                pa[:, nn * 512:(nn + 1) * 512],
                                lhsT=w_s[d][:, ch * 128:(ch + 1) * 128],
                                rhs=xT_s[d][half][:, s0:s0 + 512],
                                start=(d == 0),
                                stop=(d == NDT - 1),
                            )
                    eng.tensor_copy(
                        dst[ch][:, half * 1024:(half + 1) * 1024], pa[:, :])

            def qk_piece(w_s, dst, ch, half, nn):
                """4-matmul filler piece: one 512-col block of a projection."""
                pa = psA.tile([128, 1024], f32, name="pa", tag="pa")
                s0 = nn * 512
                for d in range(NDT):
                    nc.tensor.matmul(
                        pa[:, 0:512],
                        lhsT=w_s[d][:, ch * 128:(ch + 1) * 128],
                        rhs=xT_s[d][half][:, s0:s0 + 512],
                        start=(d == 0),
                        stop=(d == NDT - 1),
                    )
                off = half * 1024 + nn * 512
                nc.vector.tensor_copy(dst[ch][:, off:off + 512], pa[:, 0:512])

            def swap_half(src_lo, src_hi, t, sh):
                """DMA-swap one 1024-wide s-half of the lo replica into hi."""
                s0, s1 = sh * 1024, (sh + 1) * 1024
                nc.sync.dma_start(out=src_hi[t][64:128, s0:s1],
                                  in_=src_lo[t][0:64, s0:s1])
                nc.sync.dma_start(out=src_hi[t][0:64, s0:s1],
                                  in_=src_lo[t][64:128, s0:s1])

            def v_round(r):
                """V projection for j-chunks 4r..4r+3 (lead-in only)."""
                pvt = psV.tile([128, 4 * 512], f32, name="pvt", tag="pv")
                for k in range(4):
                    m = 4 * r + k
                    for d in range(NDT):
                        mh, mo = divmod(m, 8)
                        nc.tensor.matmul(
                            pvt[:, k * 512:(k + 1) * 512],
                            lhsT=xT_s[d][mh][:, mo * 128:(mo + 1) * 128],
                            rhs=wv_s[d][:, :],
                            start=(d == 0),
                            stop=(d == NDT - 1),
                        )
                for k in range(4):
                    m = 4 * r + k
                    va = v_aug[m].rearrange("p (h t) -> p h t", t=DH + 1)
                    nc.vector.tensor_copy(
                        va[:, :, 0:DH],
                        pvt[:, k * 512:(k + 1) * 512].rearrange(
                            "p (h t) -> p h t", t=DH),
                    )
                    nc.vector.memset(va[:, :, DH:DH + 1], 1.0)

            def v_piece(m):
                """4-matmul filler piece: V projection for j-chunk m."""
                pa = psA.tile([128, 1024], f32, name="pa", tag="pa")
                mh, mo = divmod(m, 8)
                for d in range(NDT):
                    nc.tensor.matmul(
                        pa[:, 0:512],
                        lhsT=xT_s[d][mh][:, mo * 128:(mo + 1) * 128],
                        rhs=wv_s[d][:, :],
                        start=(d == 0),
                        stop=(d == NDT - 1),
                    )
                va = v_aug[m].rearrange("p (h t) -> p h t", t=DH + 1)
                nc.vector.tensor_copy(
                    va[:, :, 0:DH],
                    pa[:, 0:512].rearrange("p (h t) -> p h t", t=DH),
                )
                nc.vector.memset(va[:, :, DH:DH + 1], 1.0)

            # ---- lead-in compute: Q/K for heads 0-1, V chunks 0-7 ----
            # qk halves alternate with v0 chunks so the PE never idles on a
            # PSUM-copy wait (gaps reset the clock ramp to mid pstate).
            def qk_half(w_s, dst, ch, half):
                pa = psA.tile([128, 1024], f32, name="pa", tag="pa")
                for d in range(NDT):
                    for nn in range(2):
                        s0 = nn * 512
                        nc.tensor.matmul(
                            pa[:, nn * 512:(nn + 1) * 512],
                            lhsT=w_s[d][:, ch * 128:(ch + 1) * 128],
                            rhs=xT_s[d][half][:, s0:s0 + 512],
                            start=(d == 0),
                            stop=(d == NDT - 1),
                        )
                nc.vector.tensor_copy(
                    dst[ch][:, half * 1024:(half + 1) * 1024], pa[:, :])

            pvt0 = psV.tile([128, 4 * 512], f32, name="pvt", tag="pv")

            def v_chunk_into(k, m):
                for d in range(NDT):
                    mh, mo = divmod(m, 8)
                    nc.tensor.matmul(
                        pvt0[:, k * 512:(k + 1) * 512],
                        lhsT=xT_s[d][mh][:, mo * 128:(mo + 1) * 128],
                        rhs=wv_s[d][:, :],
                        start=(d == 0),
                        stop=(d == NDT - 1),
                    )
                va = v_aug[m].rearrange("p (h t) -> p h t", t=DH + 1)
                nc.vector.tensor_copy(
                    va[:, :, 0:DH],
                    pvt0[:, k * 512:(k + 1) * 512].rearrange(
                        "p (h t) -> p h t", t=DH),
                )
                nc.vector.memset(va[:, :, DH:DH + 1], 1.0)

            qk_half(wq_s, qt_lo, 0, 0)
            v_chunk_into(0, 0)
            qk_half(wq_s, qt_lo, 0, 1)
            v_chunk_into(1, 1)
            swap_half(qt_lo, qt_hi, 0, 0)
            swap_half(qt_lo, qt_hi, 0, 1)
            qk_half(wk_s, kt_lo, 0, 0)
            v_chunk_into(2, 2)
            qk_half(wk_s, kt_lo, 0, 1)
            v_chunk_into(3, 3)
            swap_half(kt_lo, kt_hi, 0, 0)
            swap_half(kt_lo, kt_hi, 0, 1)
            v_round(1)

            # Filler schedule: {head: {jc: [thunk, ...]}}
            fillers = {h: {} for h in range(H)}

            def add_filler(h, jc, fn):
                fillers[h].setdefault(jc, []).append(fn)

            # head 0: V chunks 8..15 on even jcs. Deadline: chunk c is
            # consumed by PV(c) at loop-jc c+1; every piece lands well ahead.
            for i, m in enumerate(range(8, 16)):
                add_filler(0, 2 * i, (lambda m=m: v_piece(m)))
            # heads 1..3: Q/K projection for head pair t=h, plus swaps
            for h in range(1, 4):
                t = h  # chunk index produced during head h
                for i in range(4):
                    half, nn = divmod(i, 2)
                    add_filler(h, 2 * i,
                               (lambda t=t, half=half, nn=nn:
                                qk_piece(wq_s, qt_lo, t, half, nn)))
                add_filler(h, 7, (lambda t=t: swap_half(qt_lo, qt_hi, t, 0)))
                add_filler(h, 9, (lambda t=t: swap_half(qt_lo, qt_hi, t, 1)))
                for i in range(4):
                    half, nn = divmod(i, 2)
                    add_filler(h, 8 + 2 * i,
                               (lambda t=t, half=half, nn=nn:
                                qk_piece(wk_s, kt_lo, t, half, nn)))
                add_filler(h, 11, (lambda t=t: swap_half(kt_lo, kt_hi, t, 0)))
                add_filler(h, 15, (lambda t=t: swap_half(kt_lo, kt_hi, t, 1)))

            # ---- attention, head by head ----
            # PE order per j-chunk: scores(jc) then PV(jc-1), so the PE FIFO
            # never waits on exp(jc) before issuing scores(jc+1).
            ot = [apool.tile([128, S], bf16, name=f"ot{t}", tag=f"ot{t}")
                  for t in range(NDT)]
            recip_row = wpool.tile([1, S], f32, name="rrow", tag="rrow")
            pv_last = None
            for h in range(H):
                t, p = h // 2, h % 2
                lo_sl = slice(64 * p, 64 * p + 64)
                hi_sl = slice(64 * (1 - p), 64 * (1 - p) + 64)
                pv = psV.tile([128, 4 * 512], f32, name="pvh", tag="pv")
                ets = {}

                def pv_mms(jc):
                    for it in range(NST):
                        nc.tensor.matmul(
                            pv[0:DH + 1, it * 512:(it + 1) * 512],
                            lhsT=v_aug[jc][:, h * (DH + 1):(h + 1) * (DH + 1)],
                            rhs=ets[jc][:, it * 512:(it + 1) * 512],
                            start=(jc == 0),
                            stop=(jc == NSC - 1),
                        )

                trail = 1
                for jc in range(NSC):
                    et = epool.tile([128, S], bf16, name="et", tag="et")
                    ets[jc] = et
                    for half in range(2):
                        pa = psA.tile([128, 1024], f32, name="pa", tag="pa")
                        i0, i1 = 2 * half, 2 * half + 1
                        nc.tensor.matmul(
                            pa[:, 0:512],
                            lhsT=kt_lo[t][lo_sl, jc * 128:(jc + 1) * 128],
                            rhs=qt_lo[t][lo_sl, i0 * 512:(i0 + 1) * 512],
                        )
                        nc.tensor.matmul(
                            pa[:, 512:1024],
                            lhsT=kt_hi[t][hi_sl, jc * 128:(jc + 1) * 128],
                            rhs=qt_hi[t][hi_sl, i1 * 512:(i1 + 1) * 512],
                        )
                        nc.scalar.activation(
                            out=et[:, half * 1024:(half + 1) * 1024],
                            in_=pa[:, :],
                            func=Exp,
                            scale=SCALE,
                        )
                    # Fillers go between the score pair and PV: the filler's
                    # PSUM buffer then drains well before the scores two
                    # rotations later need it, so the exp stream never waits.
                    for fn in fillers[h].get(jc, ()):
                        fn()
                    if jc >= trail:
                        pv_mms(jc - trail)
                for jc in range(NSC - trail, NSC):
                    pv_mms(jc)

                if h < H - 1:
                    # Decouple normalization from the PE pipeline: get O_un
                    # and the denominator row out of PSUM fast, then
                    # normalize via a (128,16) reciprocal + DRAM
                    # partition-broadcast. Fully hidden under head h+1.
                    oun = spool.tile([DH + 1, S], f32, name="oun", tag="oun")
                    nc.vector.tensor_copy(oun[:, :], pv[0:DH + 1, :])
                    den128 = spool.tile([128, 16], f32, name="den128",
                                        tag="d128")
                    nc.sync.dma_start(out=den128[:, :], in_=oun[DH:DH + 1, :])
                    nc.vector.reciprocal(out=den128[:, :], in_=den128[:, :])
                    nc.sync.dma_start(out=den_dram[h, :], in_=den128[:, :])
                    bc = spool.tile([64, S], f32, name="bc", tag="bc")
                    dd = den_dram[h:h + 1, :]
                    bcast_src = bass.AP(
                        tensor=dd.tensor,
                        offset=dd.offset,
                        ap=[[0, 64]] + [list(x) for x in dd.ap[1:]],
                    )
                    nc.sync.dma_start(out=bc[:, :], in_=bcast_src)
                    nc.vector.tensor_mul(
                        ot[t][64 * p:64 * p + 64, :], oun[0:DH, :], bc[:, :])
                else:
                    pv_last = pv

            # ---- head 7 fast-path normalization (critical tail) ----
            # reciprocal of the PSUM denominator row via a (128,16) reshape
            # round trip (small SBUF-local DMAs), GpSimd partition-broadcast,
            # multiply straight out of PSUM.
            nc.vector.tensor_copy(recip_row[0:1, :], pv_last[DH:DH + 1, :])
            recip2 = spool.tile([DH + 1, S], f32, name="oun", tag="oun")
            nc.vector.reciprocal_approx_fast(out=recip2[0:1, :],
                                             in_=recip_row[0:1, :])
            bc7 = spool.tile([64, S], f32, name="bc", tag="bc")
            nc.gpsimd.partition_broadcast(bc7[:, :], recip2[0:1, :],
                                          channels=64)
            # keep the PE clock hot through the normalization bubble
            for k in range(14):
                ja = psA.tile([128, 1024], f32, name="pa", tag="pa")
                for half in range(2):
                    nc.tensor.matmul(
                        ja[:, half * 512:(half + 1) * 512],
                        lhsT=junk_sb[:, 0:128],
                        rhs=junk_sb[:, :],
                    )
            nc.vector.tensor_mul(
                ot[3][64:128, :], pv_last[0:DH, :], bc7[:, :])

            # ---- output projection (psA ping-pong so matmul groups and the
            # bias-add/copy of the previous group overlap) ----
            for ch in range(NDT):
                stage = opool.tile([128, S], f32, name="stage", tag="stage")
                for half in range(2):
                    po = psA.tile([128, 1024], f32, name="pa", tag="pa")
                    for st2 in range(2):
                        st = half * 2 + st2
                        for kt in range(NDT):
                            nc.tensor.matmul(
                                po[:, st2 * 512:(st2 + 1) * 512],
                                lhsT=wo_s[kt][:, ch * 128:(ch + 1) * 128],
                                rhs=ot[kt][:, st * 512:(st + 1) * 512],
                                start=(kt == 0),
                                stop=(kt == NDT - 1),
                            )
                    nc.vector.tensor_scalar_add(
                        out=stage[:, half * 1024:(half + 1) * 1024],
                        in0=po[:, :],
                        scalar1=bo_s[ch][:, :],
                    )
                    nc.sync.dma_start(
                        out=out[ch * 128:(ch + 1) * 128,
                                half * 1024:(half + 1) * 1024],
                        in_=stage[:, half * 1024:(half + 1) * 1024],
                    )

    nc.finalize()
    return nc


_NC_CACHE = None


def _get_nc():
    global _NC_CACHE
    if _NC_CACHE is None:
        _NC_CACHE = _build_kernel()
    return _NC_CACHE


def kernel(x, W_qkv, W_out, b_out):
    from concourse.bass_utils import run_bass_kernel_spmd

    bf16 = ml_dtypes.bfloat16

    # head-interleave and transpose the qkv weight: row 192h+{0,64,128}+c of
    # W_qkv is q/k/v row (h, c); regroup to e' = 64h+c and transpose to [d, e']
    w3 = W_qkv.reshape(H, 3, DH, D)
    wq_h = np.ascontiguousarray(w3[:, 0].reshape(INNER, D).T).astype(bf16)
    wk_h = np.ascontiguousarray(w3[:, 1].reshape(INNER, D).T).astype(bf16)
    wv_h = np.ascontiguousarray(w3[:, 2].reshape(INNER, D).T).astype(bf16)
    wo_h = np.ascontiguousarray(W_out.T).astype(bf16)  # [hc, d]
    bo_h = np.ascontiguousarray(b_out.reshape(NDT, 128, 1)).astype(np.float32)

    in_maps = []
    for b in range(N_CORES):
        xT_b = np.ascontiguousarray(x[b].T).astype(bf16)  # [d, s]
        in_maps.append({
            "xT": xT_b, "wq": wq_h, "wk": wk_h, "wv": wv_h,
            "wo": wo_h, "bo": bo_h,
        })

    nc = _get_nc()
    res = run_bass_kernel_spmd(nc, in_maps, list(range(N_CORES)))
    outs = [res.results[b]["out"].T for b in range(N_CORES)]  # [s, d] each
    return np.ascontiguousarray(np.stack(outs, axis=0)).astype(np.float32)
